# revision 31
# baseline (speedup 1.0000x reference)
"""Trainium2 Bass kernel for the ExemplarBaseline retrieval-kNN model.

Math (per batch b, fully independent across b):
    f      = data.reshape(B*T, CHW) @ W_fe + b_fe            (feature extract)
    d2     = ||f_s - f_t||^2 ; dist = d2**0.25
    sims   = exp(-c * dist)
    numers = 1e-8 + sum_{s<t} sims[s,t] * teach[s, cls]
    score  = numers**gamma / sum_cls ; score[t=0] = 1e-8

Sharding: data-parallel over the batch dim B (128) across 8 NeuronCores,
16 sequences per core.  Host pre-casts x/W to bf16 and pre-transposes x so
the device only does matmuls + a tiny fused epilogue:

  - feats^T [D, tok] = W^T @ x^T accumulated over 24 K-tiles (bf16 MMs)
  - sq[tok] = ones^T @ (fT*fT)   (diag of the Gram matrix, via PE)
  - per-b:  psum = G - 0.5*(sq_s + sq_t) = -0.5*d2  via 8 bf16 Gram MMs
            plus one fp32 "augmented" rank-2 MM that adds the sq rows
  - epilogue: d2 = max(-2*psum, 1e-12); dist = exp(0.25*ln d2);
            sims = exp(-c*dist); mask s<t; numers MM; pow via ln/exp;
            normalize; row t=0 := 1e-8
All transcendentals use only Ln/Exp (one ACT table set, no reloads).
"""

import numpy as np
import ml_dtypes

B, T, NC = 128, 128, 10
CHW, D = 3072, 1024
NCORES = 8
BL = B // NCORES          # 16 sequences per core
TOK = BL * T              # 2048 tokens per core
KT = CHW // 128           # 24 contraction tiles
DT = D // 128             # 8 feature tiles
NCHUNK = 4                # token chunks per core
CH = TOK // NCHUNK        # 512 tokens per chunk
BPC = BL // NCHUNK        # 4 sequences per chunk

EPS_NUMER = 1e-8
EPS_D2 = 1e-12

# fp8e4m3+DoubleRow feats matmul: ~2x PE rate but crashes this device
# (NRT_EXEC_UNIT_UNRECOVERABLE); bf16 is the validated configuration.
USE_FP8 = False

_NC_CACHE = {}
LAST_RESULTS = None       # BassKernelResults of the most recent run (for test.py)


def _build_bass():
    import concourse.mybir as mybir
    import concourse.tile as tile
    from concourse import bacc

    f32 = mybir.dt.float32
    bf16 = mybir.dt.bfloat16
    fp8 = mybir.dt.float8e4
    MMDT = fp8 if USE_FP8 else bf16
    AF = mybir.ActivationFunctionType
    OP = mybir.AluOpType
    PM = mybir.MatmulPerfMode

    # The ACT table-set chooser picks the FIRST set containing each function:
    # Exp -> set 0, Ln -> set 5, which makes every Ln<->Exp transition reload
    # tables (~1.3us each, ~50 reloads).  Both live together in
    # natural_log_exp_and_others; hide them from every other set so the
    # chooser lands there once.  (The map is only used for set *choice* —
    # walrus re-derives real table contents from act_info.json.)
    if not getattr(bacc, "_ln_exp_tables_patched", False):
        orig_tables = bacc.get_activation_tables

        def _patched_tables(arch):
            out = {}
            for name, funcs in orig_tables(arch).items():
                if name != "natural_log_exp_and_others":
                    funcs = funcs - {AF.Ln, AF.Exp}
                out[name] = funcs
            return out

        bacc.get_activation_tables = _patched_tables
        bacc._ln_exp_tables_patched = True

    # Bacc (not raw Bass): its compile() splits multi-sem waits into
    # EventSemaphore instructions — this walrus allows only 1 wait/inst.
    nc = bacc.Bacc("TRN2", target_bir_lowering=False)

    xT_h = nc.dram_tensor("xT", [CHW, TOK], MMDT, kind="ExternalInput")
    W_h = nc.dram_tensor("Wt", [CHW, D], MMDT, kind="ExternalInput")
    bfe_h = nc.dram_tensor("bfe", [D], f32, kind="ExternalInput")
    teach_h = nc.dram_tensor("teach", [BL, T, NC], f32, kind="ExternalInput")
    negc_h = nc.dram_tensor("negc", [128, 1], f32, kind="ExternalInput")
    gam_h = nc.dram_tensor("gam", [128, 1], f32, kind="ExternalInput")
    y_h = nc.dram_tensor("y", [BL, T, NC], f32, kind="ExternalOutput")

    xT_r = xT_h.rearrange("(kt p) n -> p kt n", p=128)     # [128, 24, 2048]
    W_r = W_h.rearrange("(kt p) d -> p kt d", p=128)       # [128, 24, 1024]
    bfe_r = bfe_h.rearrange("(dt p) -> p dt", p=128)       # [128, 8]
    teach_r = teach_h.rearrange("b s c -> s b c")          # [128, 16, 10]

    with tile.TileContext(nc) as tc:
        with (
            tc.tile_pool(name="cpool", bufs=1) as cpool,
            tc.tile_pool(name="xpool", bufs=2) as xpool,
            tc.tile_pool(name="f2pool", bufs=3) as f2pool,
            tc.tile_pool(name="wpool", bufs=2) as wpool,
            tc.tile_pool(name="spool", bufs=2) as spool,
            tc.tile_pool(name="pfpool", bufs=4, space="PSUM") as pfpool,
            tc.tile_pool(name="psqpool", bufs=1, space="PSUM") as psqpool,
            tc.tile_pool(name="pgpool", bufs=2, space="PSUM") as pgpool,
            tc.tile_pool(name="pnpool", bufs=1, space="PSUM") as pnpool,
        ):
            # ---- persistent tiles -------------------------------------
            W_sb = cpool.tile([128, KT, D], MMDT, name="W_sb")
            teach_sb = cpool.tile([128, BL, NC], f32, name="teach_sb")
            bfe_sb = cpool.tile([128, DT], f32, name="bfe_sb")
            negc_sb = cpool.tile([128, 1], f32, name="negc_sb")
            gam_sb = cpool.tile([128, 1], f32, name="gam_sb")
            eps_sb = cpool.tile([128, 1], f32, name="eps_sb")
            ones_sb = cpool.tile([128, 1], bf16, name="ones_sb")
            sqn = cpool.tile([1, TOK], f32, name="sqn")        # -0.5 * sq
            onesrow = cpool.tile([1, TOK], f32, name="onesrow")
            fT = [
                cpool.tile([128, TOK], bf16, name=f"fT{i}") for i in range(DT)
            ]

            # ---- startup DMAs: interleave W with chunk-0 x so the first
            # feats matmuls can start after ~2.4 MB instead of ~10 MB
            xc0 = xpool.tile([128, KT, CH], MMDT, name="xc")
            for k0 in range(0, KT, 6):
                nc.sync.dma_start(out=W_sb[:, k0:k0 + 6, :], in_=W_r[:, k0:k0 + 6, :])
                nc.sync.dma_start(out=xc0[:, k0:k0 + 6, :], in_=xT_r[:, k0:k0 + 6, 0:CH])
            nc.sync.dma_start(out=teach_sb, in_=teach_r)
            nc.sync.dma_start(out=bfe_sb, in_=bfe_r)
            nc.sync.dma_start(out=negc_sb, in_=negc_h[:, :])
            nc.sync.dma_start(out=gam_sb, in_=gam_h[:, :])

            nc.vector.memset(ones_sb, 1.0)
            nc.vector.memset(eps_sb, EPS_NUMER)
            nc.vector.memset(onesrow, 1.0)

            def emit_feats(c, xc):
                csl = slice(c * CH, (c + 1) * CH)
                psq = psqpool.tile([1, CH], f32, name="psq")
                for dt_i in range(DT):
                    dsl = slice(dt_i * 128, (dt_i + 1) * 128)
                    pf = pfpool.tile([128, CH], f32, name="pf")
                    if USE_FP8:
                        # fp8 DoubleRow: one MM contracts two adjacent
                        # k-tiles ([128, 2, *] slices of the existing layout)
                        for k in range(0, KT, 2):
                            nc.tensor.matmul(
                                pf, W_sb[:, k:k + 2, dsl], xc[:, k:k + 2, :],
                                start=(k == 0), stop=(k == KT - 2),
                                perf_mode=PM.DoubleRow,
                            )
                    else:
                        for k in range(KT):
                            nc.tensor.matmul(
                                pf, W_sb[:, k, dsl], xc[:, k, :],
                                start=(k == 0), stop=(k == KT - 1),
                            )
                    # evacuate psum -> fT (bf16) with per-partition bias add.
                    # On DVE (not ACT) so the scalar engine only ever runs
                    # Ln/Exp — keeps it on one ACT table set (no reloads).
                    fsl = fT[dt_i][:, csl]
                    nc.vector.tensor_scalar(
                        fsl, pf, bfe_sb[:, dt_i:dt_i + 1], None, op0=OP.add,
                    )
                    # squares for sq = sum_d f^2 (summed over d via PE)
                    f2 = f2pool.tile([128, CH], bf16, name="f2")
                    nc.vector.tensor_mul(f2, fsl, fsl)
                    nc.tensor.matmul(
                        psq, ones_sb, f2,
                        start=(dt_i == 0), stop=(dt_i == DT - 1),
                    )
                # augmented row: -0.5*sq (DVE, same reason as the evac)
                nc.vector.tensor_scalar(
                    sqn[0:1, csl], psq, -0.5, None, op0=OP.mult,
                )

            def emit_epilogue(c):
                for bi in range(BPC):
                    b = c * BPC + bi
                    tsl = slice(b * T, (b + 1) * T)
                    # psum = G - 0.5*sq_s - 0.5*sq_t = -0.5 * d2
                    pg = pgpool.tile([128, 128], f32, name="pg")
                    for dt_i in range(DT):
                        nc.tensor.matmul(
                            pg, fT[dt_i][:, tsl], fT[dt_i][:, tsl],
                            start=(dt_i == 0), stop=False,
                        )
                    # rank-1 updates: out[s,t] += -0.5*sq[s] and += -0.5*sq[t]
                    nc.tensor.matmul(
                        pg, sqn[:, tsl], onesrow[:, tsl],
                        start=False, stop=False,
                    )
                    nc.tensor.matmul(
                        pg, onesrow[:, tsl], sqn[:, tsl],
                        start=False, stop=True,
                    )
                    # dist = exp(0.25*ln(-2*psum)) = d2**0.25 straight off
                    # PSUM; sims = exp(-c*dist) with -c as per-partition
                    # scale.  Only the (masked-out) diagonal can go NaN —
                    # off-diagonal d2 ~ 2000 > 0.  All heavy epilogue
                    # elementwise work lives on ACT, not DVE, so the DVE
                    # queue stays free for the next chunk's psum evacuation.
                    lt = wpool.tile([128, 128], f32, name="lt")
                    nc.scalar.activation(lt, pg, AF.Ln, scale=-2.0)
                    dist = wpool.tile([128, 128], f32, name="dist")
                    nc.scalar.activation(dist, lt, AF.Exp, scale=0.25)
                    sims = wpool.tile([128, 128], f32, name="sims")
                    nc.scalar.activation(sims, dist, AF.Exp, scale=negc_sb)
                    # zero s >= t (kills diagonal NaNs too); on idle GpSimd.
                    # iota = t - s - 1 >= 0 keeps sims exactly where s < t.
                    simsM = wpool.tile([128, 128], f32, name="simsM")
                    nc.gpsimd.affine_select(
                        out=simsM, in_=sims,
                        compare_op=OP.is_ge, fill=0.0,
                        base=-1, pattern=[[1, 128]], channel_multiplier=-1,
                    )
                    # numers[t, cls] = sum_s simsM[s,t] * teach[s, cls]
                    pn = pnpool.tile([128, NC], f32, name="pn")
                    nc.tensor.matmul(
                        pn, simsM, teach_sb[:, b, :], start=True, stop=True,
                    )
                    # tmp = (numers + eps) ** gamma  via exp(gamma * ln(.))
                    l2 = spool.tile([128, NC], f32, name="l2")
                    nc.scalar.activation(l2, pn, AF.Ln, bias=eps_sb)
                    tmp = spool.tile([128, NC], f32, name="tmp")
                    nc.scalar.activation(tmp, l2, AF.Exp, scale=gam_sb)
                    den = spool.tile([128, 1], f32, name="den")
                    nc.vector.tensor_reduce(
                        den, tmp, axis=mybir.AxisListType.X, op=OP.add,
                    )
                    rden = spool.tile([128, 1], f32, name="rden")
                    nc.vector.reciprocal(rden, den)
                    score = spool.tile([128, NC], f32, name="score")
                    nc.vector.tensor_scalar(
                        score, tmp, rden, None, op0=OP.mult,
                    )
                    nc.vector.memset(score[0:1, :], EPS_NUMER)
                    nc.sync.dma_start(out=y_h[b], in_=score)

            # Software pipeline: emit chunk c's per-sequence epilogue AFTER
            # chunk c+1's feats matmuls, so the PE's in-order queue never
            # stalls waiting on the DVE/ACT chains the epilogue MMs consume.
            xc = xc0
            for c in range(NCHUNK):
                emit_feats(c, xc)
                if c + 1 < NCHUNK:
                    xc = xpool.tile([128, KT, CH], MMDT, name="xc")
                    nsl = slice((c + 1) * CH, (c + 2) * CH)
                    for k0 in range(0, KT, 6):
                        nc.sync.dma_start(
                            out=xc[:, k0:k0 + 6, :], in_=xT_r[:, k0:k0 + 6, nsl],
                        )
                if c > 0:
                    emit_epilogue(c - 1)
            emit_epilogue(NCHUNK - 1)

    nc.compile()
    return nc


def _get_bass():
    if "nc" not in _NC_CACHE:
        _NC_CACHE["nc"] = _build_bass()
    return _NC_CACHE["nc"]


def make_in_maps(data_t, teaching_signal_t, W_fe, b_fe, c, gamma):
    """Host-side prep: cast to the matmul dtype, transpose x, shard 8 ways."""
    import concourse.mybir as mybir
    mmdt = mybir.dt.np(mybir.dt.float8e4) if USE_FP8 else ml_dtypes.bfloat16
    x = np.asarray(data_t, np.float32).reshape(B * T, CHW)
    xbf = x.astype(mmdt)
    Wbf = np.asarray(W_fe, np.float32).astype(mmdt)
    bfe = np.ascontiguousarray(np.asarray(b_fe, np.float32).reshape(D))
    teach = np.ascontiguousarray(np.asarray(teaching_signal_t, np.float32))
    cval = np.float32(np.asarray(c, np.float32).reshape(-1)[0])
    gval = np.float32(np.asarray(gamma, np.float32).reshape(-1)[0])
    negc = np.full((128, 1), -cval, np.float32)
    gam = np.full((128, 1), gval, np.float32)

    in_maps = []
    for core in range(NCORES):
        rows = slice(core * TOK, (core + 1) * TOK)
        xT_c = np.ascontiguousarray(xbf[rows].T)          # [3072, 2048]
        in_maps.append(dict(
            xT=xT_c, Wt=Wbf, bfe=bfe,
            teach=teach[core * BL:(core + 1) * BL],
            negc=negc, gam=gam,
        ))
    return in_maps


def kernel(responses_t, data_t, teaching_signal_t, W_fe, b_fe, c, gamma):
    global LAST_RESULTS
    from concourse.bass_utils import run_bass_kernel_spmd

    in_maps = make_in_maps(data_t, teaching_signal_t, W_fe, b_fe, c, gamma)
    nc = _get_bass()
    res = run_bass_kernel_spmd(nc, in_maps, core_ids=list(range(NCORES)))
    LAST_RESULTS = res
    y = np.concatenate([r["y"] for r in res.results], axis=0)  # [128,128,10]
    return np.ascontiguousarray(y[:, :, None, :].astype(np.float32))


# revision 40
# speedup vs baseline: 1.6611x; 1.6611x over previous
"""Trainium2 Bass kernel for the ExemplarBaseline retrieval-kNN model.

Math (per batch b, fully independent across b):
    f      = data.reshape(B*T, CHW) @ W_fe + b_fe            (feature extract)
    d2     = ||f_s - f_t||^2 ; dist = d2**0.25
    sims   = exp(-c * dist)
    numers = 1e-8 + sum_{s<t} sims[s,t] * teach[s, cls]
    score  = numers**gamma / sum_cls ; score[t=0] = 1e-8

Sharding: data-parallel over the batch dim B (128) across 8 NeuronCores,
16 sequences per core.  Host pre-casts x/W to bf16 and pre-transposes x so
the device only does matmuls + a tiny fused epilogue:

  - feats^T [D, tok] = W^T @ x^T accumulated over 24 K-tiles (bf16 MMs)
  - sq[tok] = ones^T @ (fT*fT)   (diag of the Gram matrix, via PE)
  - per-b:  psum = G - 0.5*(sq_s + sq_t) = -0.5*d2  via 8 bf16 Gram MMs
            plus one fp32 "augmented" rank-2 MM that adds the sq rows
  - epilogue: d2 = max(-2*psum, 1e-12); dist = exp(0.25*ln d2);
            sims = exp(-c*dist); mask s<t; numers MM; pow via ln/exp;
            normalize; row t=0 := 1e-8
All transcendentals use only Ln/Exp (one ACT table set, no reloads).
"""

import numpy as np
import ml_dtypes

B, T, NC = 128, 128, 10
CHW, D = 3072, 1024
NCORES = 8
BL = B // NCORES          # 16 sequences per core
TOK = BL * T              # 2048 tokens per core
KT = CHW // 128           # 24 contraction tiles
DT = D // 128             # 8 feature tiles
NCHUNK = 4                # token chunks per core
CH = TOK // NCHUNK        # 512 tokens per chunk
BPC = BL // NCHUNK        # 4 sequences per chunk

EPS_NUMER = 1e-8
EPS_D2 = 1e-12

# fp8e4m3+DoubleRow feats matmul (~2x PE rate).  The fp8 path avoids all
# fp32 matmuls (fp32 LOW/HI passes interleaved with DoubleRow hard-fault
# the PE): rank-1/numers matmuls run as exact bf16 hi/lo pairs instead.
USE_FP8 = True

_NC_CACHE = {}
LAST_RESULTS = None       # BassKernelResults of the most recent run (for test.py)


def _build_bass():
    import concourse.mybir as mybir
    import concourse.tile as tile
    from concourse import bacc

    f32 = mybir.dt.float32
    bf16 = mybir.dt.bfloat16
    fp8 = mybir.dt.float8e4
    MMDT = fp8 if USE_FP8 else bf16
    AF = mybir.ActivationFunctionType
    OP = mybir.AluOpType
    PM = mybir.MatmulPerfMode

    # The ACT table-set chooser picks the FIRST set containing each function:
    # Exp -> set 0, Ln -> set 5, which makes every Ln<->Exp transition reload
    # tables (~1.3us each, ~50 reloads).  Both live together in
    # natural_log_exp_and_others; hide them from every other set so the
    # chooser lands there once.  (The map is only used for set *choice* —
    # walrus re-derives real table contents from act_info.json.)
    if not getattr(bacc, "_ln_exp_tables_patched", False):
        orig_tables = bacc.get_activation_tables

        def _patched_tables(arch):
            out = {}
            for name, funcs in orig_tables(arch).items():
                if name != "natural_log_exp_and_others":
                    funcs = funcs - {AF.Ln, AF.Exp}
                out[name] = funcs
            return out

        bacc.get_activation_tables = _patched_tables
        bacc._ln_exp_tables_patched = True

    # Bacc (not raw Bass): its compile() splits multi-sem waits into
    # EventSemaphore instructions — this walrus allows only 1 wait/inst.
    nc = bacc.Bacc("TRN2", target_bir_lowering=False)

    xT_h = nc.dram_tensor("xT", [CHW, TOK], MMDT, kind="ExternalInput")
    W_h = nc.dram_tensor("Wt", [CHW, D], MMDT, kind="ExternalInput")
    bfe_h = nc.dram_tensor("bfe", [D], f32, kind="ExternalInput")
    # fp8 path: no fp32 matmuls anywhere (suspected fp32 LOW/HI pass +
    # DoubleRow interaction hard-faults the PE).  teach arrives as a bf16
    # hi/lo pair so the numers matmul stays exact in bf16.
    if USE_FP8:
        teach_h = nc.dram_tensor("teach", [2, BL, T, NC], bf16,
                                 kind="ExternalInput")
        teach_r = teach_h.rearrange("two b s c -> s two b c")  # [128,2,16,10]
    else:
        teach_h = nc.dram_tensor("teach", [BL, T, NC], f32, kind="ExternalInput")
        teach_r = teach_h.rearrange("b s c -> s b c")          # [128, 16, 10]
    negc_h = nc.dram_tensor("negc", [128, 1], f32, kind="ExternalInput")
    gam_h = nc.dram_tensor("gam", [128, 1], f32, kind="ExternalInput")
    y_h = nc.dram_tensor("y", [BL, T, NC], f32, kind="ExternalOutput")

    xT_r = xT_h.rearrange("(kt p) n -> p kt n", p=128)     # [128, 24, 2048]
    W_r = W_h.rearrange("(kt p) d -> p kt d", p=128)       # [128, 24, 1024]
    bfe_r = bfe_h.rearrange("(dt p) -> p dt", p=128)       # [128, 8]

    with tile.TileContext(nc) as tc:
        with (
            tc.tile_pool(name="cpool", bufs=1) as cpool,
            tc.tile_pool(name="xpool", bufs=2) as xpool,
            tc.tile_pool(name="f2pool", bufs=3) as f2pool,
            tc.tile_pool(name="wpool", bufs=2) as wpool,
            tc.tile_pool(name="spool", bufs=2) as spool,
            tc.tile_pool(name="pfpool", bufs=4, space="PSUM") as pfpool,
            tc.tile_pool(name="psqpool", bufs=1, space="PSUM") as psqpool,
            tc.tile_pool(name="pgpool", bufs=2, space="PSUM") as pgpool,
            tc.tile_pool(name="pnpool", bufs=1, space="PSUM") as pnpool,
        ):
            # ---- persistent tiles -------------------------------------
            W_sb = cpool.tile([128, KT, D], MMDT, name="W_sb")
            if USE_FP8:
                teach_sb = cpool.tile([128, 2, BL, NC], bf16, name="teach_sb")
            else:
                teach_sb = cpool.tile([128, BL, NC], f32, name="teach_sb")
            bfe_sb = cpool.tile([128, DT], f32, name="bfe_sb")
            negc_sb = cpool.tile([128, 1], f32, name="negc_sb")
            gam_sb = cpool.tile([128, 1], f32, name="gam_sb")
            eps_sb = cpool.tile([128, 1], f32, name="eps_sb")
            ones_sb = cpool.tile([128, 1], bf16, name="ones_sb")
            AUGDT = bf16 if USE_FP8 else f32
            if USE_FP8:
                # hi/lo split of -0.5*sq so the rank-1 updates stay bf16
                # (exact to ~2^-16 relative) with no fp32 matmuls
                sqn_lo = cpool.tile([1, TOK], bf16, name="sqn_lo")
            sqn = cpool.tile([1, TOK], AUGDT, name="sqn")      # -0.5 * sq
            onesrow = cpool.tile([1, TOK], AUGDT, name="onesrow")
            fT = [
                cpool.tile([128, TOK], bf16, name=f"fT{i}") for i in range(DT)
            ]

            # ---- startup DMAs: interleave W with chunk-0 x so the first
            # feats matmuls can start after ~2.4 MB instead of ~10 MB
            xc0 = xpool.tile([128, KT, CH], MMDT, name="xc")
            for k0 in range(0, KT, 6):
                nc.sync.dma_start(out=W_sb[:, k0:k0 + 6, :], in_=W_r[:, k0:k0 + 6, :])
                nc.sync.dma_start(out=xc0[:, k0:k0 + 6, :], in_=xT_r[:, k0:k0 + 6, 0:CH])
            nc.sync.dma_start(out=teach_sb, in_=teach_r)
            nc.sync.dma_start(out=bfe_sb, in_=bfe_r)
            nc.sync.dma_start(out=negc_sb, in_=negc_h[:, :])
            nc.sync.dma_start(out=gam_sb, in_=gam_h[:, :])

            nc.vector.memset(ones_sb, 1.0)
            nc.vector.memset(eps_sb, EPS_NUMER)
            nc.vector.memset(onesrow, 1.0)

            def emit_feats(c, xc):
                csl = slice(c * CH, (c + 1) * CH)
                psq = psqpool.tile([1, CH], f32, name="psq")
                for dt_i in range(DT):
                    dsl = slice(dt_i * 128, (dt_i + 1) * 128)
                    pf = pfpool.tile([128, CH], f32, name="pf")
                    if USE_FP8:
                        # fp8 DoubleRow: one MM contracts two adjacent
                        # k-tiles ([128, 2, *] slices of the existing layout)
                        for k in range(0, KT, 2):
                            nc.tensor.matmul(
                                pf, W_sb[:, k:k + 2, dsl], xc[:, k:k + 2, :],
                                start=(k == 0), stop=(k == KT - 2),
                                perf_mode=PM.DoubleRow,
                            )
                    else:
                        for k in range(KT):
                            nc.tensor.matmul(
                                pf, W_sb[:, k, dsl], xc[:, k, :],
                                start=(k == 0), stop=(k == KT - 1),
                            )
                    # evacuate psum -> fT (bf16) with per-partition bias add.
                    # On DVE (not ACT) so the scalar engine only ever runs
                    # Ln/Exp — keeps it on one ACT table set (no reloads).
                    fsl = fT[dt_i][:, csl]
                    nc.vector.tensor_scalar(
                        fsl, pf, bfe_sb[:, dt_i:dt_i + 1], None, op0=OP.add,
                    )
                    # squares for sq = sum_d f^2 (summed over d via PE)
                    f2 = f2pool.tile([128, CH], bf16, name="f2")
                    nc.vector.tensor_mul(f2, fsl, fsl)
                    nc.tensor.matmul(
                        psq, ones_sb, f2,
                        start=(dt_i == 0), stop=(dt_i == DT - 1),
                    )
                # augmented row: -0.5*sq (DVE, same reason as the evac)
                if USE_FP8:
                    sqf = wpool.tile([1, CH], f32, name="sqf")
                    nc.vector.tensor_scalar(
                        sqf, psq, -0.5, None, op0=OP.mult,
                    )
                    nc.vector.tensor_copy(sqn[0:1, csl], sqf)       # hi
                    nc.vector.tensor_sub(sqn_lo[0:1, csl], sqf, sqn[0:1, csl])
                else:
                    nc.vector.tensor_scalar(
                        sqn[0:1, csl], psq, -0.5, None, op0=OP.mult,
                    )

            def emit_epilogue(c):
                for bi in range(BPC):
                    b = c * BPC + bi
                    tsl = slice(b * T, (b + 1) * T)
                    # psum = G - 0.5*sq_s - 0.5*sq_t = -0.5 * d2
                    pg = pgpool.tile([128, 128], f32, name="pg")
                    for dt_i in range(DT):
                        nc.tensor.matmul(
                            pg, fT[dt_i][:, tsl], fT[dt_i][:, tsl],
                            start=(dt_i == 0), stop=False,
                        )
                    # rank-1 updates: out[s,t] += -0.5*sq[s] and += -0.5*sq[t]
                    nc.tensor.matmul(
                        pg, sqn[:, tsl], onesrow[:, tsl],
                        start=False, stop=False,
                    )
                    if USE_FP8:
                        nc.tensor.matmul(
                            pg, sqn_lo[:, tsl], onesrow[:, tsl],
                            start=False, stop=False,
                        )
                        nc.tensor.matmul(
                            pg, onesrow[:, tsl], sqn_lo[:, tsl],
                            start=False, stop=False,
                        )
                    nc.tensor.matmul(
                        pg, onesrow[:, tsl], sqn[:, tsl],
                        start=False, stop=True,
                    )
                    # dist = exp(0.25*ln(-2*psum)) = d2**0.25 straight off
                    # PSUM; sims = exp(-c*dist) with -c as per-partition
                    # scale.  Only the (masked-out) diagonal can go NaN —
                    # off-diagonal d2 ~ 2000 > 0.  All heavy epilogue
                    # elementwise work lives on ACT, not DVE, so the DVE
                    # queue stays free for the next chunk's psum evacuation.
                    lt = wpool.tile([128, 128], f32, name="lt")
                    nc.scalar.activation(lt, pg, AF.Ln, scale=-2.0)
                    dist = wpool.tile([128, 128], f32, name="dist")
                    nc.scalar.activation(dist, lt, AF.Exp, scale=0.25)
                    SIMDT = bf16 if USE_FP8 else f32
                    sims = wpool.tile([128, 128], SIMDT, name="sims")
                    nc.scalar.activation(sims, dist, AF.Exp, scale=negc_sb)
                    # zero s >= t (kills diagonal NaNs too); on idle GpSimd.
                    # iota = t - s - 1 >= 0 keeps sims exactly where s < t.
                    simsM = wpool.tile([128, 128], SIMDT, name="simsM")
                    nc.gpsimd.affine_select(
                        out=simsM, in_=sims,
                        compare_op=OP.is_ge, fill=0.0,
                        base=-1, pattern=[[1, 128]], channel_multiplier=-1,
                    )
                    # numers[t, cls] = sum_s simsM[s,t] * teach[s, cls]
                    pn = pnpool.tile([128, NC], f32, name="pn")
                    if USE_FP8:
                        nc.tensor.matmul(
                            pn, simsM, teach_sb[:, 0, b, :],
                            start=True, stop=False,
                        )
                        nc.tensor.matmul(
                            pn, simsM, teach_sb[:, 1, b, :],
                            start=False, stop=True,
                        )
                    else:
                        nc.tensor.matmul(
                            pn, simsM, teach_sb[:, b, :], start=True, stop=True,
                        )
                    # tmp = (numers + eps) ** gamma  via exp(gamma * ln(.))
                    l2 = spool.tile([128, NC], f32, name="l2")
                    nc.scalar.activation(l2, pn, AF.Ln, bias=eps_sb)
                    tmp = spool.tile([128, NC], f32, name="tmp")
                    nc.scalar.activation(tmp, l2, AF.Exp, scale=gam_sb)
                    den = spool.tile([128, 1], f32, name="den")
                    nc.vector.tensor_reduce(
                        den, tmp, axis=mybir.AxisListType.X, op=OP.add,
                    )
                    rden = spool.tile([128, 1], f32, name="rden")
                    nc.vector.reciprocal(rden, den)
                    score = spool.tile([128, NC], f32, name="score")
                    nc.vector.tensor_scalar(
                        score, tmp, rden, None, op0=OP.mult,
                    )
                    nc.vector.memset(score[0:1, :], EPS_NUMER)
                    nc.sync.dma_start(out=y_h[b], in_=score)

            # Software pipeline: emit chunk c's per-sequence epilogue AFTER
            # chunk c+1's feats matmuls, so the PE's in-order queue never
            # stalls waiting on the DVE/ACT chains the epilogue MMs consume.
            xc = xc0
            for c in range(NCHUNK):
                emit_feats(c, xc)
                if c + 1 < NCHUNK:
                    xc = xpool.tile([128, KT, CH], MMDT, name="xc")
                    nsl = slice((c + 1) * CH, (c + 2) * CH)
                    for k0 in range(0, KT, 6):
                        nc.sync.dma_start(
                            out=xc[:, k0:k0 + 6, :], in_=xT_r[:, k0:k0 + 6, nsl],
                        )
                if c > 0:
                    emit_epilogue(c - 1)
            emit_epilogue(NCHUNK - 1)

    nc.compile()
    return nc


def _get_bass():
    if "nc" not in _NC_CACHE:
        _NC_CACHE["nc"] = _build_bass()
    return _NC_CACHE["nc"]


def make_in_maps(data_t, teaching_signal_t, W_fe, b_fe, c, gamma):
    """Host-side prep: cast to the matmul dtype, transpose x, shard 8 ways."""
    import concourse.mybir as mybir
    mmdt = mybir.dt.np(mybir.dt.float8e4) if USE_FP8 else ml_dtypes.bfloat16
    x = np.asarray(data_t, np.float32).reshape(B * T, CHW)
    xbf = x.astype(mmdt)
    Wbf = np.asarray(W_fe, np.float32).astype(mmdt)
    bfe = np.ascontiguousarray(np.asarray(b_fe, np.float32).reshape(D))
    teach = np.ascontiguousarray(np.asarray(teaching_signal_t, np.float32))
    if USE_FP8:
        th = teach.astype(ml_dtypes.bfloat16)
        tl = (teach - th.astype(np.float32)).astype(ml_dtypes.bfloat16)
        teach2 = np.stack([th, tl])          # [2, B, T, NC] bf16
    cval = np.float32(np.asarray(c, np.float32).reshape(-1)[0])
    gval = np.float32(np.asarray(gamma, np.float32).reshape(-1)[0])
    negc = np.full((128, 1), -cval, np.float32)
    gam = np.full((128, 1), gval, np.float32)

    in_maps = []
    for core in range(NCORES):
        rows = slice(core * TOK, (core + 1) * TOK)
        xT_c = np.ascontiguousarray(xbf[rows].T)          # [3072, 2048]
        if USE_FP8:
            tc_ = np.ascontiguousarray(teach2[:, core * BL:(core + 1) * BL])
        else:
            tc_ = teach[core * BL:(core + 1) * BL]
        in_maps.append(dict(
            xT=xT_c, Wt=Wbf, bfe=bfe, teach=tc_,
            negc=negc, gam=gam,
        ))
    return in_maps


def kernel(responses_t, data_t, teaching_signal_t, W_fe, b_fe, c, gamma):
    global LAST_RESULTS
    from concourse.bass_utils import run_bass_kernel_spmd

    in_maps = make_in_maps(data_t, teaching_signal_t, W_fe, b_fe, c, gamma)
    nc = _get_bass()
    res = run_bass_kernel_spmd(nc, in_maps, core_ids=list(range(NCORES)))
    LAST_RESULTS = res
    y = np.concatenate([r["y"] for r in res.results], axis=0)  # [128,128,10]
    return np.ascontiguousarray(y[:, :, None, :].astype(np.float32))
